# revision 13
# baseline (speedup 1.0000x reference)
"""Trainium2 Bass kernel: Conv2d(1,1,5x5,SAME) + FastLIF + FastLI temporal scan.

Input  x[T=256, 1, H=512, W=512] fp32, conv kernel [1,1,5,5] fp32.
Computation:
    c = conv2d_same(x, K)              (per-timestep, time-invariant weights)
    per t: v = 0.85*s1 + c_t; spk = (v>=2); s1 = v - 2*spk; s2 = 0.9*s2 + spk
    out[t] = s2
Sharding: H split across 8 cores (64 rows each); identical SPMD program per
core, halo rows shipped host-side, no collectives.

V3 design (per core):
- Layout: partitions p = 64*wh + h, free = w % 256 (128x256 pixels/step).
- Conv on PE in f16 (1 cyc/row, mantissa >= tf32): slab loaded wh-stacked
  with 2-col w-halos; 5 dw-banded [128,128] stationaries + 1 packed-halo
  [40,128] stationary. Matmuls grouped 4 timesteps per PSUM tile
  [128,4,256] (6 matmul instructions per 4 steps, 2 banks/tile, 4 tiles =
  all 8 banks = 16-step PE lookahead).
- DVE per step: v = 0.85*s1 + C (STT from PSUM); q = (v>=2)*ALPHA_LI^-j
  (dual-scalar TS); s1' = v - 2*ALPHA_LI^j * q (STT). j = index in batch.
- Pool per step: running sum P_j = P_{j-1} + q_j (single TT); per batch one
  TS computes the carry a2^TB * P_{TB-1} (exact fp32, s2 state never
  quantized).
- ACT per step: s2 out = a2^j * P_j (Copy w/ scale) written f16 (output
  quantization ~5e-4 rel, state unaffected); |v-2| -> fp8 flag history.
- Host patches pixels whose fp8 min_t |v-2| <= eps exactly in fp32 (the
  spike pattern is only precision-sensitive within ~1e-3 of threshold).
- DMA: inputs f16 (x slab + pre-shifted halos), outputs f16 s2 + fp8 flags
  packed [T/2,128,512] so every DMA run is >= 512B.
"""

import sys

import numpy as np

if "/opt/trn_rl_repo" not in sys.path:
    sys.path.insert(0, "/opt/trn_rl_repo")

T_FULL = 256
H = 512
W = 512
NCORES = 8
HSH = H // NCORES          # 64 rows per core
WHF = W // 2               # 256 free elems per partition
SLABW = WHF + 4            # 260: w-half plus 2-col halo each side
ALPHA_LIF = 0.85
V_TH = 2.0
ALPHA_LI = 0.9
FLAG_EPS = 4.0e-3          # host patches pixels with min_t |v-2| <= FLAG_EPS

_PROGRAM_CACHE = {}


def build_program(T=T_FULL, TB=16, flag=True, level=4, TS_G=2):
    from contextlib import ExitStack

    import concourse.bass as bass
    import concourse.tile as tile
    from concourse import bacc, mybir

    f32 = mybir.dt.float32
    f16 = mybir.dt.float16
    f8 = mybir.dt.float8e4
    Alu = mybir.AluOpType
    Act = mybir.ActivationFunctionType
    assert T % TB == 0 and TB % TS_G == 0 and TB % 2 == 0

    nc = bacc.Bacc(
        "TRN2",
        target_bir_lowering=False,
        debug=False,
        enable_asserts=False,
        num_devices=NCORES,
    )
    x_sh = nc.dram_tensor("x_sh", [T, HSH, W], f16, kind="ExternalInput").ap()
    halo_sh = nc.dram_tensor("halo_sh", [T, 40, WHF], f16, kind="ExternalInput").ap()
    stat_m = nc.dram_tensor("stat_m", [5, 128, 128], f16, kind="ExternalInput").ap()
    stat_h = nc.dram_tensor("stat_h", [40, 128], f16, kind="ExternalInput").ap()
    out_sh = nc.dram_tensor("out_sh", [T, HSH, W], f16, kind="ExternalOutput").ap()
    if flag:
        d8_sh = nc.dram_tensor(
            "d8_sh", [T // 2, 128, 2 * WHF], f8, kind="ExternalOutput"
        ).ap()

    a2 = float(ALPHA_LI)

    with tile.TileContext(nc) as tc, ExitStack() as ctx:
        const = ctx.enter_context(tc.tile_pool(name="const", bufs=1))
        slabp = ctx.enter_context(tc.tile_pool(name="slab", bufs=2))
        halop = ctx.enter_context(tc.tile_pool(name="halo", bufs=2))
        cpool = ctx.enter_context(
            tc.tile_pool(name="cpsum", bufs=8, space=bass.MemorySpace.PSUM)
        )
        vpoolA = ctx.enter_context(tc.tile_pool(name="vA", bufs=8))
        vpoolB = ctx.enter_context(tc.tile_pool(name="vB", bufs=8))
        qpoolA = ctx.enter_context(tc.tile_pool(name="qA", bufs=8))
        qpoolB = ctx.enter_context(tc.tile_pool(name="qB", bufs=8))
        s1poolA = ctx.enter_context(tc.tile_pool(name="s1A", bufs=3))
        s1poolB = ctx.enter_context(tc.tile_pool(name="s1B", bufs=3))
        pbpool = ctx.enter_context(tc.tile_pool(name="Pb", bufs=6))
        hpool = ctx.enter_context(tc.tile_pool(name="hist", bufs=2))
        crpool = ctx.enter_context(tc.tile_pool(name="carry", bufs=2))
        if flag:
            dpool = ctx.enter_context(tc.tile_pool(name="d8", bufs=2))

        stm = const.tile([128, 5, 128], f16)
        nc.sync.dma_start(stm[:], stat_m.transpose([1, 0, 2]))
        sth = const.tile([40, 128], f16)
        nc.sync.dma_start(sth[:], stat_h[:])
        zero = const.tile([128, WHF], f32)
        nc.vector.memset(zero[:], 0.0)
        if flag:
            # non-Copy ACT funcs need the bias as a per-partition AP
            biasq = const.tile([128, 1], f32)
            nc.vector.memset(biasq[:], -V_TH)

        s1_prev = [zero[:, 0:128], zero[:, 128:WHF]]
        carry = zero[:]          # a2 * s2 entering the batch (fp32 exact)
        HV = 128                 # column split point for the two DVE chains

        def load_batch(tb):
            t0 = tb * TB
            slab = slabp.tile([128, TB, SLABW], f16, tag="slab")
            # wh0 rows: cols 2..260 <- w [0, 258); pad cols 0:2 (w -2,-1)
            # wh1 rows: cols 0..258 <- w [254, 512); pad cols 258:260
            nc.gpsimd.memset(slab[0:64, :, 0:2], 0.0)
            nc.gpsimd.memset(slab[64:128, :, SLABW - 2 : SLABW], 0.0)
            nc.sync.dma_start(
                slab[0:64, :, 2:SLABW],
                x_sh[t0 : t0 + TB, :, 0 : WHF + 2].transpose([1, 0, 2]),
            )
            nc.sync.dma_start(
                slab[64:128, :, 0 : SLABW - 2],
                x_sh[t0 : t0 + TB, :, WHF - 2 : W].transpose([1, 0, 2]),
            )
            halo = halop.tile([40, TB, WHF], f16, tag="halo")
            nc.sync.dma_start(halo[:], halo_sh[t0 : t0 + TB].transpose([1, 0, 2]))
            return slab, halo

        # Software pipeline: chain B (cols [HV:WHF)) runs one full timestep
        # behind chain A (cols [0:HV)). Interleaving A(ti) with B(ti-1) on
        # DVE gives every same-engine semaphore a full op-pair of
        # propagation time, so the serial v->q->s1 recurrence never stalls.
        nxt = load_batch(0)
        batch = {}   # bi -> (t0, hist, d8h, slab, halo)
        Crec = {}    # step -> (C tile, lane)
        Pbs = {}     # step -> Pb ap
        sA_prev = zero[:, 0:HV]
        sB_prev = zero[:, HV:WHF]

        def emit_out_dmas(b):
            t0, hist, d8h = batch[b][0], batch[b][1], batch[b][2]
            for wh in (0, 1):
                nc.sync.dma_start(
                    out_sh[t0 : t0 + TB, :, WHF * wh : WHF * wh + WHF].transpose(
                        [1, 0, 2]
                    ),
                    hist[64 * wh : 64 * wh + 64, :, :],
                )
            if flag:
                nc.sync.dma_start(
                    d8_sh[t0 // 2 : (t0 + TB) // 2].transpose([1, 0, 2]), d8h[:]
                )
            del batch[b]

        def emit_B(tj):
            """All chain-B work for step tj plus the full-width closers."""
            jB = tj % TB
            bB = tj // TB
            CB, tlB = Crec.pop(tj)
            vB = vpoolB.tile([128, WHF - HV], f32)
            qB = qpoolB.tile([128, WHF - HV], f32)
            s1B = s1poolB.tile([128, WHF - HV], f32)
            return vB, qB, s1B, jB, bB, CB, tlB

        for ti in range(T):
            j = ti % TB
            bi = ti // TB
            if j == 0:
                slab, halo = nxt
                if bi + 1 < T // TB:
                    nxt = load_batch(bi + 1)
                hist = hpool.tile([128, TB, WHF], f16)
                d8h = dpool.tile([128, TB // 2, 2 * WHF], f8, name="d8h") if flag else None
                batch[bi] = (ti, hist, d8h, slab, halo)
            if ti % TS_G == 0:
                C = cpool.tile([128, TS_G, WHF], f32)
                sl = batch[bi][3]
                hl = batch[bi][4]
                g0 = j - (j % TS_G)
                for k, dwo in enumerate((-2, -1, 0, 1, 2)):
                    nc.tensor.matmul(
                        C[:, :, :],
                        stm[:, dwo + 2, :],
                        sl[:, g0 : g0 + TS_G, dwo + 2 : dwo + 2 + WHF],
                        start=(k == 0),
                        stop=False,
                    )
                nc.tensor.matmul(
                    C[:, :, :], sth[:], hl[:, g0 : g0 + TS_G, :],
                    start=False, stop=True,
                )
                for k in range(TS_G):
                    Crec[ti + k] = (C, k)
            tj = ti - 1
            hasB = tj >= 0
            if hasB:
                vB, qB, s1B, jB, bB, CB, tlB = emit_B(tj)
            CA, tlA = Crec[ti]
            # --- DVE: interleave A(ti) with B(ti-1) ---
            vA = vpoolA.tile([128, HV], f32)
            nc.vector.scalar_tensor_tensor(
                vA[:], sA_prev, ALPHA_LIF, CA[:, tlA, 0:HV], Alu.mult, Alu.add
            )
            if hasB:
                nc.vector.scalar_tensor_tensor(
                    vB[:], sB_prev, ALPHA_LIF, CB[:, tlB, HV:WHF],
                    Alu.mult, Alu.add,
                )
            qA = qpoolA.tile([128, HV], f32)
            nc.vector.tensor_scalar(
                qA[:], vA[:], V_TH, a2 ** (-j), Alu.is_ge, Alu.mult
            )
            if hasB:
                nc.vector.tensor_scalar(
                    qB[:], vB[:], V_TH, a2 ** (-jB), Alu.is_ge, Alu.mult
                )
            s1A = s1poolA.tile([128, HV], f32)
            nc.vector.scalar_tensor_tensor(
                s1A[:], qA[:], -V_TH * a2**j, vA[:], Alu.mult, Alu.add
            )
            if hasB:
                nc.vector.scalar_tensor_tensor(
                    s1B[:], qB[:], -V_TH * a2**jB, vB[:], Alu.mult, Alu.add
                )
                sB_prev = s1B[:]
            sA_prev = s1A[:]
            # --- ACT: threshold-distance history ---
            if flag:
                o8 = (j % 2) * WHF
                nc.scalar.activation(
                    batch[bi][2][:, j // 2, o8 : o8 + HV], vA[:],
                    Act.Abs, bias=biasq[:], scale=1.0,
                )
                if hasB:
                    o8B = (jB % 2) * WHF
                    nc.scalar.activation(
                        batch[bB][2][:, jB // 2, o8B + HV : o8B + WHF], vB[:],
                        Act.Abs, bias=biasq[:], scale=1.0,
                    )
            # --- Pool: running spike sums (B half first, then carry, A) ---
            if hasB:
                prevB = carry if jB == 0 else Pbs[tj - 1]
                nc.gpsimd.tensor_tensor(
                    Pbs[tj][:, HV:WHF], prevB[:, HV:WHF], qB[:], Alu.add
                )
                if j == 0:
                    # batch of tj complete: carry = a2^TB * P_{TB-1}
                    cr = crpool.tile([128, WHF], f32)
                    nc.gpsimd.tensor_scalar(
                        cr[:], Pbs[tj], a2**TB, None, Alu.mult
                    )
                    carry = cr[:]
            Pb = pbpool.tile([128, WHF], f32)
            prevA = carry if j == 0 else Pbs[ti - 1]
            nc.gpsimd.tensor_tensor(Pb[:, 0:HV], prevA[:, 0:HV], qA[:], Alu.add)
            Pbs[ti] = Pb[:]
            # --- ACT: s2 = a2^j * P_j (f16 out), then batch-b DMAs ---
            if hasB:
                nc.scalar.mul(batch[bB][1][:, jB, :], Pbs[tj], a2**jB)
                if j == 0:
                    emit_out_dmas(bB)
                if tj - 2 in Pbs:
                    del Pbs[tj - 2]
        # --- tail: finish chain B for the last step ---
        tj = T - 1
        vB, qB, s1B, jB, bB, CB, tlB = emit_B(tj)
        nc.vector.scalar_tensor_tensor(
            vB[:], sB_prev, ALPHA_LIF, CB[:, tlB, HV:WHF], Alu.mult, Alu.add
        )
        nc.vector.tensor_scalar(
            qB[:], vB[:], V_TH, a2 ** (-jB), Alu.is_ge, Alu.mult
        )
        if flag:
            o8B = (jB % 2) * WHF
            nc.scalar.activation(
                batch[bB][2][:, jB // 2, o8B + HV : o8B + WHF], vB[:],
                Act.Abs, bias=biasq[:], scale=1.0,
            )
        prevB = Pbs[tj - 1]
        nc.gpsimd.tensor_tensor(
            Pbs[tj][:, HV:WHF], prevB[:, HV:WHF], qB[:], Alu.add
        )
        nc.scalar.mul(batch[bB][1][:, jB, :], Pbs[tj], a2**jB)
        emit_out_dmas(bB)
    nc.compile()
    return nc


def _get_program(T, TB=16, flag=True, level=4):
    key = (T, TB, flag, level)
    if key not in _PROGRAM_CACHE:
        _PROGRAM_CACHE[key] = build_program(T, TB, flag, level)
    return _PROGRAM_CACHE[key]


def make_stats(K):
    """Banded stationaries: stat_m [5,128,128] (rows 64wh+r, cols 64wh+h'),
    stat_h [40,128] (rows (dw,wh,j) matching halo_sh)."""
    stat_m = np.zeros((5, 128, 128), np.float16)
    for dw in range(5):
        for wh in (0, 1):
            for hp in range(HSH):
                for dh in range(5):
                    r = hp + dh - 2
                    if 0 <= r < HSH:
                        stat_m[dw, 64 * wh + r, 64 * wh + hp] = K[dh, dw]
    # halo taps: j in {0,1,2,3} <-> shard rows {-2,-1,64,65}
    taps = {0: [(0, 0)], 1: [(1, 0), (0, 1)], 2: [(63, 3), (62, 4)], 3: [(63, 4)]}
    stat_h = np.zeros((40, 128), np.float16)
    for dw in range(5):
        for wh in (0, 1):
            for j, tl in taps.items():
                for hp, dh in tl:
                    stat_h[dw * 8 + wh * 4 + j, 64 * wh + hp] = K[dh, dw]
    return stat_m, stat_h


def make_shards(xs):
    """xs [T, H, W] f16 -> per-core (x_sh [T,64,512], halo_sh [T,40,256])."""
    T = xs.shape[0]
    shards = []
    for k in range(NCORES):
        h0 = k * HSH
        x_sh = np.ascontiguousarray(xs[:, h0 : h0 + HSH, :])
        halo = np.zeros((T, 40, WHF), np.float16)
        rows = [h0 - 2, h0 - 1, h0 + HSH, h0 + HSH + 1]
        for dw in range(5):
            dwo = dw - 2
            for wh in (0, 1):
                for j, hr in enumerate(rows):
                    if not (0 <= hr < H):
                        continue
                    wlo = WHF * wh + dwo
                    whi = wlo + WHF
                    slo = max(wlo, 0)
                    shi = min(whi, W)
                    halo[:, dw * 8 + wh * 4 + j, slo - wlo : slo - wlo + shi - slo] = (
                        xs[:, hr, slo:shi]
                    )
        shards.append((x_sh, halo))
    return shards


def lif_scan_pixels(c, T):
    """Exact fp32 reference scan for c[T, F] -> out[T, F]."""
    F = c.shape[1]
    s1 = np.zeros(F, np.float32)
    s2 = np.zeros(F, np.float32)
    out = np.empty((T, F), np.float32)
    a1 = np.float32(ALPHA_LIF)
    a2 = np.float32(ALPHA_LI)
    th = np.float32(V_TH)
    for t in range(T):
        v = a1 * s1 + c[t]
        spk = (v >= th).astype(np.float32)
        s1 = v - spk * th
        s2 = a2 * s2 + spk
        out[t] = s2
    return out


def patch_output(out, xs32, K, d8_list, eps=FLAG_EPS):
    """Recompute flagged pixels exactly in fp32 and patch them in-place.

    out: [T, H, W] f32 device result; xs32: [T, H, W] fp32 input; K: [5,5];
    d8_list: per-core [T//2, 128, 512] fp8 |v - 2| histories.
    """
    T = out.shape[0]
    ys, xw = [], []
    for k, d8 in enumerate(d8_list):
        d = d8.astype(np.float32).reshape(T // 2, 128, 2, WHF)
        near = d.min(axis=(0, 2)) <= eps  # [128, 256]
        p, wp = np.nonzero(near)
        wh = p // 64
        hh = k * HSH + (p % 64)
        ww = wh * WHF + wp
        ys.append(hh)
        xw.append(ww)
    hh = np.concatenate(ys)
    ww = np.concatenate(xw)
    n = hh.size
    if n == 0:
        return 0
    # exact conv series for flagged pixels
    xp = np.pad(xs32, ((0, 0), (2, 2), (2, 2)))
    c = np.zeros((T, n), np.float32)
    for dh in range(5):
        for dw in range(5):
            c += np.float32(K[dh, dw]) * xp[:, hh + dh, ww + dw]
    out[:, hh, ww] = lif_scan_pixels(c, T)
    return n


def run_on_hw(x, kern, T=T_FULL, TB=16, flag=True, patch=True):
    from concourse.bass_utils import run_bass_kernel_spmd

    xs32 = np.ascontiguousarray(np.asarray(x, dtype=np.float32)[:, 0])  # [T, H, W]
    xs = xs32.astype(np.float16)
    K = np.asarray(kern, dtype=np.float32)[0, 0]  # [5, 5]
    stat_m, stat_h = make_stats(K)
    in_maps = [
        {"x_sh": sh, "halo_sh": halo, "stat_m": stat_m, "stat_h": stat_h}
        for sh, halo in make_shards(xs)
    ]
    nc = _get_program(T, TB, flag)
    res = run_bass_kernel_spmd(nc, in_maps, list(range(NCORES)))
    out = np.concatenate(
        [res.results[k]["out_sh"].astype(np.float32) for k in range(NCORES)], axis=1
    )
    npatched = 0
    if flag and patch:
        d8_list = [res.results[k]["d8_sh"] for k in range(NCORES)]
        npatched = patch_output(out, xs32, K, d8_list)
    return out[:, None, :, :].astype(np.float32), res, npatched


def kernel(**inputs):
    out, _, _ = run_on_hw(inputs["x"], inputs["kernel"])
    return out


# revision 16
# speedup vs baseline: 1.0823x; 1.0823x over previous
"""Trainium2 Bass kernel: Conv2d(1,1,5x5,SAME) + FastLIF + FastLI temporal scan.

Input  x[T=256, 1, H=512, W=512] fp32, conv kernel [1,1,5,5] fp32.
Computation:
    c = conv2d_same(x, K)              (per-timestep, time-invariant weights)
    per t: v = 0.85*s1 + c_t; spk = (v>=2); s1 = v - 2*spk; s2 = 0.9*s2 + spk
    out[t] = s2
Sharding: H split across 8 cores (64 rows each); identical SPMD program per
core, halo rows shipped host-side, no collectives.

V3 design (per core):
- Layout: partitions p = 64*wh + h, free = w % 256 (128x256 pixels/step).
- Conv on PE in f16 (1 cyc/row, mantissa >= tf32): slab loaded wh-stacked
  with 2-col w-halos; 5 dw-banded [128,128] stationaries + 1 packed-halo
  [40,128] stationary. Matmuls grouped 4 timesteps per PSUM tile
  [128,4,256] (6 matmul instructions per 4 steps, 2 banks/tile, 4 tiles =
  all 8 banks = 16-step PE lookahead).
- DVE per step: v = 0.85*s1 + C (STT from PSUM); q = (v>=2)*ALPHA_LI^-j
  (dual-scalar TS); s1' = v - 2*ALPHA_LI^j * q (STT). j = index in batch.
- Pool per step: running sum P_j = P_{j-1} + q_j (single TT); per batch one
  TS computes the carry a2^TB * P_{TB-1} (exact fp32, s2 state never
  quantized).
- ACT per step: s2 out = a2^j * P_j (Copy w/ scale) written f16 (output
  quantization ~5e-4 rel, state unaffected); |v-2| -> fp8 flag history.
- Host patches pixels whose fp8 min_t |v-2| <= eps exactly in fp32 (the
  spike pattern is only precision-sensitive within ~1e-3 of threshold).
- DMA: inputs f16 (x slab + pre-shifted halos), outputs f16 s2 + fp8 flags
  packed [T/2,128,512] so every DMA run is >= 512B.
"""

import sys

import numpy as np

if "/opt/trn_rl_repo" not in sys.path:
    sys.path.insert(0, "/opt/trn_rl_repo")

T_FULL = 256
H = 512
W = 512
NCORES = 8
HSH = H // NCORES          # 64 rows per core
WHF = W // 2               # 256 free elems per partition
SLABW = WHF + 4            # 260: w-half plus 2-col halo each side
ALPHA_LIF = 0.85
V_TH = 2.0
ALPHA_LI = 0.9
FLAG_EPS = 4.0e-3          # host patches pixels with min_t |v-2| <= FLAG_EPS

_PROGRAM_CACHE = {}


def build_program(T=T_FULL, TB=16, flag=True, level=4, TS_G=2):
    from contextlib import ExitStack

    import concourse.bass as bass
    import concourse.tile as tile
    from concourse import bacc, mybir

    f32 = mybir.dt.float32
    f16 = mybir.dt.float16
    f8 = mybir.dt.float8e4
    Alu = mybir.AluOpType
    Act = mybir.ActivationFunctionType
    assert T % TB == 0 and TB % TS_G == 0 and TB % 2 == 0

    nc = bacc.Bacc(
        "TRN2",
        target_bir_lowering=False,
        debug=False,
        enable_asserts=False,
        num_devices=NCORES,
    )
    x_sh = nc.dram_tensor("x_sh", [T, HSH, W], f16, kind="ExternalInput").ap()
    halo_sh = nc.dram_tensor("halo_sh", [T, 40, WHF], f16, kind="ExternalInput").ap()
    stat_m = nc.dram_tensor("stat_m", [5, 128, 128], f16, kind="ExternalInput").ap()
    stat_h = nc.dram_tensor("stat_h", [40, 128], f16, kind="ExternalInput").ap()
    out_sh = nc.dram_tensor("out_sh", [T, HSH, W], f16, kind="ExternalOutput").ap()
    if flag:
        d8_sh = nc.dram_tensor(
            "d8_sh", [T // 2, 128, 2 * WHF], f8, kind="ExternalOutput"
        ).ap()

    a2 = float(ALPHA_LI)

    with tile.TileContext(nc) as tc, ExitStack() as ctx:
        const = ctx.enter_context(tc.tile_pool(name="const", bufs=1))
        slabp = ctx.enter_context(tc.tile_pool(name="slab", bufs=2))
        halop = ctx.enter_context(tc.tile_pool(name="halo", bufs=2))
        cpool = ctx.enter_context(
            tc.tile_pool(name="cpsum", bufs=8, space=bass.MemorySpace.PSUM)
        )
        vpool = ctx.enter_context(tc.tile_pool(name="v", bufs=6))
        qpool = ctx.enter_context(tc.tile_pool(name="q", bufs=6))
        s1pool = ctx.enter_context(tc.tile_pool(name="s1", bufs=2))
        pbpool = ctx.enter_context(tc.tile_pool(name="Pb", bufs=4))
        hpool = ctx.enter_context(tc.tile_pool(name="hist", bufs=2))
        crpool = ctx.enter_context(tc.tile_pool(name="carry", bufs=2))
        if flag:
            dpool = ctx.enter_context(tc.tile_pool(name="d8", bufs=2))

        stm = const.tile([128, 5, 128], f16)
        nc.sync.dma_start(stm[:], stat_m.transpose([1, 0, 2]))
        sth = const.tile([40, 128], f16)
        nc.sync.dma_start(sth[:], stat_h[:])
        zero = const.tile([128, WHF], f32)
        nc.vector.memset(zero[:], 0.0)
        if flag:
            # non-Copy ACT funcs need the bias as a per-partition AP
            biasq = const.tile([128, 1], f32)
            nc.vector.memset(biasq[:], -V_TH)

        s1_prev = zero[:]
        carry = zero[:]          # a2 * s2 entering the batch (fp32 exact)

        def load_batch(tb):
            t0 = tb * TB
            slab = slabp.tile([128, TB, SLABW], f16, tag="slab")
            # wh0 rows: cols 2..260 <- w [0, 258); pad cols 0:2 (w -2,-1)
            # wh1 rows: cols 0..258 <- w [254, 512); pad cols 258:260
            nc.gpsimd.memset(slab[0:64, :, 0:2], 0.0)
            nc.gpsimd.memset(slab[64:128, :, SLABW - 2 : SLABW], 0.0)
            nc.sync.dma_start(
                slab[0:64, :, 2:SLABW],
                x_sh[t0 : t0 + TB, :, 0 : WHF + 2].transpose([1, 0, 2]),
            )
            nc.sync.dma_start(
                slab[64:128, :, 0 : SLABW - 2],
                x_sh[t0 : t0 + TB, :, WHF - 2 : W].transpose([1, 0, 2]),
            )
            halo = halop.tile([40, TB, WHF], f16, tag="halo")
            nc.sync.dma_start(halo[:], halo_sh[t0 : t0 + TB].transpose([1, 0, 2]))
            return slab, halo

        nxt = load_batch(0)
        for tb in range(T // TB):
            t0 = tb * TB
            slab, halo = nxt
            if tb + 1 < T // TB:
                nxt = load_batch(tb + 1)
            hist = hpool.tile([128, TB, WHF], f16)
            if flag:
                d8h = dpool.tile([128, TB // 2, 2 * WHF], f8)
            for tp in range(TB // TS_G):
                C = cpool.tile([128, TS_G, WHF], f32)
                tg0 = tp * TS_G
                for j, dwo in enumerate((-2, -1, 0, 1, 2)):
                    nc.tensor.matmul(
                        C[:, :, :],
                        stm[:, dwo + 2, :],
                        slab[:, tg0 : tg0 + TS_G, dwo + 2 : dwo + 2 + WHF],
                        start=(j == 0),
                        stop=False,
                    )
                nc.tensor.matmul(
                    C[:, :, :],
                    sth[:],
                    halo[:, tg0 : tg0 + TS_G, :],
                    start=False,
                    stop=True,
                )
                for tl in range(TS_G):
                    ti = tg0 + tl
                    if ti % 2 == 0:
                        vpair = vpool.tile([128, 2 * WHF], f32, name="vpair")
                    v = vpair[:, (ti % 2) * WHF : (ti % 2 + 1) * WHF]
                    nc.vector.scalar_tensor_tensor(
                        v, s1_prev, ALPHA_LIF, C[:, tl, :], Alu.mult, Alu.add
                    )
                    if level <= 1:
                        s1_prev = v
                        continue
                    # q = (v >= 2) * a2^-ti   (spike, pre-scaled for the
                    # running-sum form of the FastLI readout)
                    q = qpool.tile([128, WHF], f32)
                    nc.vector.tensor_scalar(
                        q[:], v, V_TH, a2 ** (-ti), Alu.is_ge, Alu.mult
                    )
                    s1n = s1pool.tile([128, WHF], f32)
                    nc.vector.scalar_tensor_tensor(
                        s1n[:], q[:], -V_TH * a2**ti, v, Alu.mult, Alu.add
                    )
                    s1_prev = s1n[:]
                    if flag and level >= 4 and ti % 2 == 1:
                        # one ACT op covers both steps of the pair; layout
                        # matches the packed fp8 history exactly
                        nc.scalar.activation(
                            d8h[:, ti // 2, :],
                            vpair[:],
                            Act.Abs,
                            bias=biasq[:],
                            scale=1.0,
                        )
                    if level <= 2:
                        continue
                    # P_j = P_{j-1} + q  (Pool running sum, fp32,
                    # per-step tiles so ACT's read of P_{j-1} never blocks
                    # the next Pool write)
                    Pb = pbpool.tile([128, WHF], f32)
                    prev = carry if ti == 0 else Pb_prev
                    nc.gpsimd.tensor_tensor(Pb[:], prev, q[:], Alu.add)
                    Pb_prev = Pb[:]
                    # s2 = a2^ti * P_ti  (ACT rescale, f16 output)
                    nc.scalar.mul(hist[:, ti, :], Pb[:], a2**ti)
            if level >= 3:
                # carry = a2^TB * P_{TB-1} = a2 * s2_last  (exact fp32)
                cr = crpool.tile([128, WHF], f32)
                nc.gpsimd.tensor_scalar(
                    cr[:], Pb_prev, a2**TB, None, Alu.mult
                )
                carry = cr[:]
                for wh in (0, 1):
                    nc.sync.dma_start(
                        out_sh[t0 : t0 + TB, :, WHF * wh : WHF * wh + WHF].transpose(
                            [1, 0, 2]
                        ),
                        hist[64 * wh : 64 * wh + 64, :, :],
                    )
            if flag and level >= 4:
                nc.sync.dma_start(
                    d8_sh[t0 // 2 : (t0 + TB) // 2].transpose([1, 0, 2]), d8h[:]
                )
    nc.compile()
    return nc


def _get_program(T, TB=16, flag=True, level=4):
    key = (T, TB, flag, level)
    if key not in _PROGRAM_CACHE:
        _PROGRAM_CACHE[key] = build_program(T, TB, flag, level)
    return _PROGRAM_CACHE[key]


def make_stats(K):
    """Banded stationaries: stat_m [5,128,128] (rows 64wh+r, cols 64wh+h'),
    stat_h [40,128] (rows (dw,wh,j) matching halo_sh)."""
    stat_m = np.zeros((5, 128, 128), np.float16)
    for dw in range(5):
        for wh in (0, 1):
            for hp in range(HSH):
                for dh in range(5):
                    r = hp + dh - 2
                    if 0 <= r < HSH:
                        stat_m[dw, 64 * wh + r, 64 * wh + hp] = K[dh, dw]
    # halo taps: j in {0,1,2,3} <-> shard rows {-2,-1,64,65}
    taps = {0: [(0, 0)], 1: [(1, 0), (0, 1)], 2: [(63, 3), (62, 4)], 3: [(63, 4)]}
    stat_h = np.zeros((40, 128), np.float16)
    for dw in range(5):
        for wh in (0, 1):
            for j, tl in taps.items():
                for hp, dh in tl:
                    stat_h[dw * 8 + wh * 4 + j, 64 * wh + hp] = K[dh, dw]
    return stat_m, stat_h


def make_shards(xs):
    """xs [T, H, W] f16 -> per-core (x_sh [T,64,512], halo_sh [T,40,256])."""
    T = xs.shape[0]
    shards = []
    for k in range(NCORES):
        h0 = k * HSH
        x_sh = np.ascontiguousarray(xs[:, h0 : h0 + HSH, :])
        halo = np.zeros((T, 40, WHF), np.float16)
        rows = [h0 - 2, h0 - 1, h0 + HSH, h0 + HSH + 1]
        for dw in range(5):
            dwo = dw - 2
            for wh in (0, 1):
                for j, hr in enumerate(rows):
                    if not (0 <= hr < H):
                        continue
                    wlo = WHF * wh + dwo
                    whi = wlo + WHF
                    slo = max(wlo, 0)
                    shi = min(whi, W)
                    halo[:, dw * 8 + wh * 4 + j, slo - wlo : slo - wlo + shi - slo] = (
                        xs[:, hr, slo:shi]
                    )
        shards.append((x_sh, halo))
    return shards


def lif_scan_pixels(c, T):
    """Exact fp32 reference scan for c[T, F] -> out[T, F]."""
    F = c.shape[1]
    s1 = np.zeros(F, np.float32)
    s2 = np.zeros(F, np.float32)
    out = np.empty((T, F), np.float32)
    a1 = np.float32(ALPHA_LIF)
    a2 = np.float32(ALPHA_LI)
    th = np.float32(V_TH)
    for t in range(T):
        v = a1 * s1 + c[t]
        spk = (v >= th).astype(np.float32)
        s1 = v - spk * th
        s2 = a2 * s2 + spk
        out[t] = s2
    return out


def patch_output(out, xs32, K, d8_list, eps=FLAG_EPS):
    """Recompute flagged pixels exactly in fp32 and patch them in-place.

    out: [T, H, W] f32 device result; xs32: [T, H, W] fp32 input; K: [5,5];
    d8_list: per-core [T//2, 128, 512] fp8 |v - 2| histories.
    """
    T = out.shape[0]
    ys, xw = [], []
    for k, d8 in enumerate(d8_list):
        d = d8.astype(np.float32).reshape(T // 2, 128, 2, WHF)
        near = d.min(axis=(0, 2)) <= eps  # [128, 256]
        p, wp = np.nonzero(near)
        wh = p // 64
        hh = k * HSH + (p % 64)
        ww = wh * WHF + wp
        ys.append(hh)
        xw.append(ww)
    hh = np.concatenate(ys)
    ww = np.concatenate(xw)
    n = hh.size
    if n == 0:
        return 0
    # exact conv series for flagged pixels
    xp = np.pad(xs32, ((0, 0), (2, 2), (2, 2)))
    c = np.zeros((T, n), np.float32)
    for dh in range(5):
        for dw in range(5):
            c += np.float32(K[dh, dw]) * xp[:, hh + dh, ww + dw]
    out[:, hh, ww] = lif_scan_pixels(c, T)
    return n


def run_on_hw(x, kern, T=T_FULL, TB=16, flag=True, patch=True):
    from concourse.bass_utils import run_bass_kernel_spmd

    xs32 = np.ascontiguousarray(np.asarray(x, dtype=np.float32)[:, 0])  # [T, H, W]
    xs = xs32.astype(np.float16)
    K = np.asarray(kern, dtype=np.float32)[0, 0]  # [5, 5]
    stat_m, stat_h = make_stats(K)
    in_maps = [
        {"x_sh": sh, "halo_sh": halo, "stat_m": stat_m, "stat_h": stat_h}
        for sh, halo in make_shards(xs)
    ]
    nc = _get_program(T, TB, flag)
    res = run_bass_kernel_spmd(nc, in_maps, list(range(NCORES)))
    out = np.concatenate(
        [res.results[k]["out_sh"].astype(np.float32) for k in range(NCORES)], axis=1
    )
    npatched = 0
    if flag and patch:
        d8_list = [res.results[k]["d8_sh"] for k in range(NCORES)]
        npatched = patch_output(out, xs32, K, d8_list)
    return out[:, None, :, :].astype(np.float32), res, npatched


def kernel(**inputs):
    out, _, _ = run_on_hw(inputs["x"], inputs["kernel"])
    return out
